# revision 62
# baseline (speedup 1.0000x reference)
"""Multi-head attention (B=2, S=2048, D=1024, H=16, dk=64) on 8 TRN2 cores.

Sharding: core c handles batch b = c//4 and head group hg = c%4 (4 heads,
256 head-dims).  Each core computes Q/K/V projections for its head slice,
attention for its 4 heads, and a partial output projection against the
matching 256-row slice of Wo.  The host sums the 4 partials per batch.

Key compaction (exact): the key mask is per (batch, key) and zeroes the
softmax weight of masked keys exactly, so the host gathers the unmasked
key/value rows (~1000 of 2048 per batch) and pads to SK=1152.  K/V
projection, scores, Exp and PV shrink by ~44% with bit-identical math
(pad columns get a -300 additive bias, exp -> 0).

Math simplifications (exact up to fp rounding):
  - bk dropped: softmax(q.(k+bk)) == softmax(q.k + const_per_row) == softmax(q.k)
  - bv dropped on device: attn rows sum to 1, so ctx = attn@V0 + bv; the
    bv term contributes the constant row bv@Wo.T, added on host with bo.
  - scores computed TRANSPOSED (S^T[k,q] = K.Q^T) so the key mask is a
    per-partition bias folded into the Exp activation, and P^T feeds the
    PV matmul directly (no on-chip transposes anywhere).
  - V gets a ones-column appended (stationary M=65) so the softmax
    denominators fall out of the PV matmul for free as output row 64.

Precision: the whole pipeline runs bf16 into fp32 PSUM accumulators
(measured ~3e-3 max rel err vs the 2e-2 gate).  Output partials are
written bf16 and summed on the host in fp32.

Scheduling: activations/weights are host-permuted to [128, ...] flat
layouts and fetched with a few long-line DMAs (more/smaller DMA chunks
measurably regress: each dma_start costs ~1us of sync-engine descriptor
work); softmax normalization runs off the PE stream entirely (DVE moves
the denominator row to partition 0, ACT computes 1/d = exp(-ln d) --
Ln/Exp share an activation table with the softmax Exp, and the plain
Reciprocal DVE op is ~8 cycles/element on a single partition -- then
GpSimd partition_broadcast fans it out and DVE multiplies); the output
projection is emitted in thirds as soon as the needed ctx halves exist,
plugging the attention-group boundary stalls with PE work.
"""

import os
import ml_dtypes
import numpy as np

from contextlib import ExitStack

import concourse.bass as bass
import concourse.mybir as mybir
import concourse.tile as tile
from concourse import bacc
from concourse.bass_utils import run_bass_kernel_spmd

F32 = mybir.dt.float32
BF16 = mybir.dt.bfloat16
F8 = mybir.dt.float8e4

D_MODEL = 1024
S = 2048          # query length
SK = 1152         # compacted+padded key length (9 tiles of 128)
KT = SK // 128    # key tiles
BATCH = 2
N_CORES = 8
HEADS_PER_CORE = 4
DK = 64
DO = HEADS_PER_CORE * DK  # 256 head-dims per core
MASK_BIAS = -300.0

AF = mybir.ActivationFunctionType
ALU = mybir.AluOpType


def build_program() -> bass.Bass:
    nc = bacc.Bacc("TRN2", target_bir_lowering=False, debug=False,
                   num_devices=N_CORES)

    # activations / weights arrive host-permuted: [(t p) s] -> [p (t s)]
    xq2 = nc.declare_dram_parameter("xq2", [128, 8 * S], BF16, isOutput=False)
    xk2 = nc.declare_dram_parameter("xk2", [128, 8 * SK], BF16, isOutput=False)
    xv2 = nc.declare_dram_parameter("xv2", [128, 8 * SK], BF16, isOutput=False)
    wq2 = nc.declare_dram_parameter("wq2", [128, 8 * DO], BF16, isOutput=False)
    wk2 = nc.declare_dram_parameter("wk2", [128, 8 * DO], BF16, isOutput=False)
    wv2 = nc.declare_dram_parameter("wv2", [128, 8 * DO], BF16, isOutput=False)
    woT = nc.declare_dram_parameter("woT", [DO, D_MODEL], BF16, isOutput=False)
    onesv = nc.declare_dram_parameter("onesv", [128, KT * HEADS_PER_CORE],
                                      BF16, isOutput=False)
    bq2 = nc.declare_dram_parameter("bq2", [128, 2], F32, isOutput=False)
    maskb = nc.declare_dram_parameter("maskb", [128, KT], F32, isOutput=False)
    out = nc.declare_dram_parameter("out", [S, D_MODEL], BF16, isOutput=True)
    out3 = out.rearrange("(j p) o -> p j o", p=128)

    with tile.TileContext(nc) as tc, ExitStack() as ctx, \
            nc.allow_low_precision(reason="bf16 pipeline, 2e-2 tolerance"):
        consts = ctx.enter_context(tc.tile_pool(name="consts", bufs=1))
        big = ctx.enter_context(tc.tile_pool(name="big", bufs=1))
        ppool = ctx.enter_context(tc.tile_pool(name="ppool", bufs=2))
        opool = ctx.enter_context(tc.tile_pool(name="opool", bufs=2))
        ps_mm = ctx.enter_context(tc.tile_pool(name="ps_mm", bufs=3, space="PSUM"))
        ps_ctx = ctx.enter_context(tc.tile_pool(name="ps_ctx", bufs=5, space="PSUM"))

        # ---- prefetch: one long-line DMA per tensor ----
        wv_sb = consts.tile([128, 8, DO], BF16)
        nc.sync.dma_start(wv_sb, wv2.rearrange("p (t d) -> p t d", t=8))
        xv_sb = big.tile([128, 8, SK], BF16)
        xv3 = xv2.rearrange("p (t s) -> p t s", t=8)
        for lo, hi in ((0, 384), (384, 768), (768, SK)):
            nc.sync.dma_start(xv_sb[:, :, lo:hi], xv3[:, :, lo:hi])
        wk_sb = consts.tile([128, 8, DO], BF16)
        nc.sync.dma_start(wk_sb, wk2.rearrange("p (t d) -> p t d", t=8))
        wq_sb = consts.tile([128, 8, DO], BF16)
        nc.sync.dma_start(wq_sb, wq2.rearrange("p (t d) -> p t d", t=8))
        xk_sb = big.tile([128, 8, SK], BF16)
        xk3 = xk2.rearrange("p (t s) -> p t s", t=8)
        xq_sb = big.tile([128, 8, S], BF16)
        xq3 = xq2.rearrange("p (t s) -> p t s", t=8)
        nc.sync.dma_start(xk_sb[:, :, 0:SK // 2], xk3[:, :, 0:SK // 2])
        nc.sync.dma_start(xq_sb[:, :, 0:512], xq3[:, :, 0:512])
        nc.sync.dma_start(xk_sb[:, :, SK // 2:SK], xk3[:, :, SK // 2:SK])
        for lo, hi in ((512, 1024), (1024, 1536), (1536, 2048)):
            nc.sync.dma_start(xq_sb[:, :, lo:hi], xq3[:, :, lo:hi])
        bq_sb = consts.tile([128, 2], F32)
        nc.sync.dma_start(bq_sb, bq2[:, :])
        mask_sb = consts.tile([128, KT], F32)
        nc.sync.dma_start(mask_sb, maskb[:, :])

        # ---- persistent activations ----
        qT_sb = big.tile([128, 2, S], BF16)   # Q^T: head-pair tiles on partitions
        kT_sb = big.tile([128, 2, SK], BF16)
        # normalized ctx^T per head-pair, [128 c, S] (head hh at partition
        # 64*hh) so the output projection contracts 128 partitions at once
        ctx_tiles = [big.tile([128, S], BF16, name=f"ctxT{hp}", tag=f"ctxT{hp}")
                     for hp in range(2)]
        # V tiles with the ones column (written once by a single DMA)
        v_sb = big.tile([128, KT, HEADS_PER_CORE, DK + 1], BF16)
        nc.sync.dma_start(
            v_sb[:, :, :, DK:DK + 1],
            onesv.rearrange("p (t h o) -> p t h o", t=KT, o=1))

        # ---- V projection: V[s, do] tiles ----
        for st in range(KT):
            ps = ps_mm.tile([128, 512], F32, name="ps_v", tag="mm")
            for di in range(8):
                nc.tensor.matmul(
                    ps[:, :DO],
                    lhsT=xv_sb[:, di, st * 128:(st + 1) * 128],
                    rhs=wv_sb[:, di, :],
                    start=(di == 0), stop=(di == 7))
            nc.vector.tensor_copy(
                out=v_sb[:, st, :, 0:DK],
                in_=ps[:, :DO].rearrange("p (h d) -> p h d", h=HEADS_PER_CORE))

        # ---- K^T projection (SK wide) ----
        for off, width in ((0, 512), (512, 512), (1024, 128)):
            for dt_ in range(2):
                ps = ps_mm.tile([128, 512], F32, name="ps_k", tag="mm")
                for di in range(8):
                    nc.tensor.matmul(
                        ps[:, :width],
                        lhsT=wk_sb[:, di, dt_ * 128:(dt_ + 1) * 128],
                        rhs=xk_sb[:, di, off:off + width],
                        start=(di == 0), stop=(di == 7))
                nc.vector.tensor_copy(out=kT_sb[:, dt_, off:off + width],
                                      in_=ps[:, :width])

        # ---- Q^T projection ----
        for sc in range(4):
            for dt_ in range(2):
                ps = ps_mm.tile([128, 512], F32, name="ps_q", tag="mm")
                for di in range(8):
                    nc.tensor.matmul(
                        ps, lhsT=wq_sb[:, di, dt_ * 128:(dt_ + 1) * 128],
                        rhs=xq_sb[:, di, sc * 512:(sc + 1) * 512],
                        start=(di == 0), stop=(di == 7))
                nc.scalar.activation(out=qT_sb[:, dt_, sc * 512:(sc + 1) * 512],
                                     in_=ps, func=AF.Identity,
                                     bias=bq_sb[:, dt_:dt_ + 1], scale=1.0)

        # ---- output projection weights (needed mid-kernel) ----
        wo_tiles = []
        for hp in range(2):
            wt = consts.tile([128, D_MODEL], BF16, name=f"wo{hp}", tag=f"wo{hp}")
            nc.sync.dma_start(wt, woT[hp * 128:(hp + 1) * 128, :])
            wo_tiles.append(wt)

        def attention_group(hp, qh, prev_tail=None):
            """The last key-tile's PV and the normalize chain are deferred
            into the returned closure, which the NEXT group emits right
            after its first scores batch: the PE crosses the group boundary
            straight into fresh scores while the previous tail drains on
            ACT/GpSimd/DVE."""
            ctx_ps = [[ps_ctx.tile([128, 512], F32, name="ctx_ps", tag="ctx")
                       for _ in range(2)] for _ in range(2)]
            state = {}

            def do_pv(st, pts):
                for hh in range(2):
                    for qc in range(2):
                        nc.tensor.matmul(
                            ctx_ps[hh][qc][0:DK + 1, :],
                            lhsT=v_sb[:, st, 2 * hp + hh, :],
                            rhs=pts[(hh, qc)],
                            start=(st == 0), stop=(st == KT - 1))

            def normalize():
                # off the PE stream: ACT computes 1/d = exp(-ln d) straight
                # from the PSUM denominator row (partition 64 -> 0; Ln/Exp
                # share an activation table with the softmax Exp), GpSimd
                # broadcasts partition 0 to 64 partitions, DVE multiplies.
                dens, rsbs = [], []
                for hh in range(2):
                    for qc in range(2):
                        den0 = ppool.tile([1, 512], F32, name="den0",
                                          tag="rp", bufs=4)
                        lnd = ppool.tile([1, 512], F32, name="lnd",
                                         tag="lnd", bufs=4)
                        nc.scalar.activation(out=lnd[0:1, :],
                                             in_=ctx_ps[hh][qc][DK:DK + 1, :],
                                             func=AF.Ln, scale=1.0)
                        nc.scalar.activation(out=den0[0:1, :],
                                             in_=lnd[0:1, :],
                                             func=AF.Exp, scale=-1.0)
                        dens.append(den0)
                for i in range(4):
                    r_sb = ppool.tile([64, 512], F32, name="r_sb",
                                      tag="r_sb", bufs=4)
                    nc.gpsimd.partition_broadcast(r_sb[:, :], dens[i][0:1, :])
                    rsbs.append(r_sb)
                for hh in range(2):
                    for qc in range(2):
                        qoff = qh * 1024 + qc * 512
                        nc.vector.tensor_tensor(
                            ctx_tiles[hp][64 * hh:64 * hh + 64,
                                          qoff:qoff + 512],
                            ctx_ps[hh][qc][0:DK, :],
                            rsbs[2 * hh + qc][:, :],
                            ALU.mult)

            for st in range(KT):
                pts = {}
                for hh in range(2):
                    p0 = 64 * hh
                    for qc in range(2):
                        qoff = qh * 1024 + qc * 512
                        sps = ps_mm.tile([128, 512], F32, name="sps", tag="mm")
                        nc.tensor.matmul(
                            sps,
                            lhsT=kT_sb[p0:p0 + 64, hp,
                                       st * 128:(st + 1) * 128],
                            rhs=qT_sb[p0:p0 + 64, hp, qoff:qoff + 512],
                            start=True, stop=True,
                            tile_position=(p0, 0))
                        pt = ppool.tile([128, 512], BF16, name="pt",
                                        tag="pT", bufs=8)
                        nc.scalar.activation(
                            out=pt, in_=sps, func=AF.Exp,
                            bias=mask_sb[:, st:st + 1], scale=0.125)
                        pts[(hh, qc)] = pt
                if st == 0 and prev_tail is not None:
                    prev_tail()
                if st < KT - 1:
                    do_pv(st, pts)
                else:
                    state["last_pts"] = pts

            def tail():
                do_pv(KT - 1, state["last_pts"])
                normalize()

            return tail

        def out_proj(jlo, n=8, tail=False):
            batches = [(j, 2) for j in range(jlo, jlo + n, 2)]
            if tail:
                batches = batches[:-1] + [(jlo + n - 2, 1), (jlo + n - 1, 1)]
            for j, w in batches:
                o_sb = opool.tile([128, 2, D_MODEL], BF16, name="o_sb", tag="o")
                for jj in range(w):
                    for oc in range(2):
                        ps = ps_mm.tile([128, 512], F32, name="ps_o", tag="mm")
                        so = j + jj
                        for hp in range(2):
                            nc.tensor.matmul(
                                ps,
                                lhsT=ctx_tiles[hp][:, so * 128:(so + 1) * 128],
                                rhs=wo_tiles[hp][:, oc * 512:(oc + 1) * 512],
                                start=(hp == 0), stop=(hp == 1))
                        nc.vector.tensor_copy(
                            out=o_sb[:, jj, oc * 512:(oc + 1) * 512], in_=ps)
                nc.sync.dma_start(out3[:, j:j + w, :], o_sb[:, 0:w, :])

        # query half 0: both head pairs, then its output projection rows;
        # half of it is deferred to plug the group-3/group-4 boundary
        t1 = attention_group(0, 0)
        t2 = attention_group(1, 0, prev_tail=t1)
        t3 = attention_group(0, 1, prev_tail=t2)
        out_proj(0, 4)
        t4 = attention_group(1, 1, prev_tail=t3)
        t4()
        out_proj(4, 4)
        out_proj(8, tail=True)

    nc.finalize()
    return nc


_NC_CACHE: dict = {}
LAST_RESULTS = None


def _get_program() -> bass.Bass:
    if "nc" not in _NC_CACHE:
        _NC_CACHE["nc"] = build_program()
    return _NC_CACHE["nc"]


def _flat(a, t=8):
    """[ (t p) s ] -> [ p (t s) ] host permutation, contiguous bf16."""
    tp, s_ = a.shape
    return np.ascontiguousarray(
        a.reshape(t, 128, s_).transpose(1, 0, 2).reshape(128, t * s_))


def make_in_maps(query, key_, value, mask, Wq, bq, Wk, Wv, Wo):
    bf16 = ml_dtypes.bfloat16
    in_maps = []
    # per-batch key compaction (exact: masked keys have softmax weight 0)
    kc, vc, nk = [], [], []
    for b in range(BATCH):
        idx = np.flatnonzero(mask[b, 0, 0])
        n = len(idx)
        assert n <= SK, f"mask keeps {n} keys > SK={SK}; raise SK and rebuild"
        kcb = np.zeros((SK, D_MODEL), np.float32)
        vcb = np.zeros((SK, D_MODEL), np.float32)
        kcb[:n] = key_[b][idx]
        vcb[:n] = value[b][idx]
        kc.append(kcb)
        vc.append(vcb)
        nk.append(n)
    for c in range(N_CORES):
        b, hg = divmod(c, 4)
        sl = slice(hg * DO, (hg + 1) * DO)
        pos = np.arange(SK).reshape(KT, 128).T
        maskbd = np.where(pos < nk[b], np.float32(0.0),
                          np.float32(MASK_BIAS)).astype(np.float32)
        in_maps.append({
            "xq2": _flat(query[b].T.astype(bf16)),
            "xk2": _flat(kc[b].T.astype(bf16)),
            "xv2": _flat(vc[b].T.astype(bf16)),
            "wq2": _flat(Wq[sl, :].T.astype(bf16)),
            "wk2": _flat(Wk[sl, :].T.astype(bf16)),
            "wv2": _flat(Wv[sl, :].T.astype(bf16)),
            "woT": np.ascontiguousarray(Wo[:, sl].T.astype(bf16)),
            "bq2": np.ascontiguousarray(bq[sl].reshape(2, 128).T,
                                        dtype=np.float32),
            "maskb": maskbd,
            "onesv": np.ones((128, KT * HEADS_PER_CORE), bf16),
        })
    return in_maps


def kernel(query, key_, value, mask, Wq, bq, Wk, bk, Wv, bv, Wo, bo):
    global LAST_RESULTS
    query = np.asarray(query, dtype=np.float32)
    key_ = np.asarray(key_, dtype=np.float32)
    value = np.asarray(value, dtype=np.float32)
    mask = np.asarray(mask)
    nc = _get_program()
    in_maps = make_in_maps(query, key_, value, mask,
                           np.asarray(Wq), np.asarray(bq), np.asarray(Wk),
                           np.asarray(Wv), np.asarray(Wo))
    res = run_bass_kernel_spmd(nc, in_maps, list(range(N_CORES)))
    LAST_RESULTS = res
    # host-side unshard: sum head-group partials, add bias correction row
    corr = (np.asarray(bv, dtype=np.float32) @ np.asarray(Wo, dtype=np.float32).T
            + np.asarray(bo, dtype=np.float32))
    out = np.zeros((BATCH, S, D_MODEL), np.float32)
    for c in range(N_CORES):
        out[c // 4] += np.asarray(res.results[c]["out"], dtype=np.float32)
    out += corr[None, None, :]
    return out


# revision 63
# speedup vs baseline: 1.0050x; 1.0050x over previous
"""Multi-head attention (B=2, S=2048, D=1024, H=16, dk=64) on 8 TRN2 cores.

Sharding: core c handles batch b = c//4 and head group hg = c%4 (4 heads,
256 head-dims).  Each core computes Q/K/V projections for its head slice,
attention for its 4 heads, and a partial output projection against the
matching 256-row slice of Wo.  The host sums the 4 partials per batch.

Key compaction (exact): the key mask is per (batch, key) and zeroes the
softmax weight of masked keys exactly, so the host gathers the unmasked
key/value rows (~1000 of 2048 per batch) and pads to SK=1152.  K/V
projection, scores, Exp and PV shrink by ~44% with bit-identical math
(pad columns get a -300 additive bias, exp -> 0).

Math simplifications (exact up to fp rounding):
  - bk dropped: softmax(q.(k+bk)) == softmax(q.k + const_per_row) == softmax(q.k)
  - bv dropped on device: attn rows sum to 1, so ctx = attn@V0 + bv; the
    bv term contributes the constant row bv@Wo.T, added on host with bo.
  - scores computed TRANSPOSED (S^T[k,q] = K.Q^T) so the key mask is a
    per-partition bias folded into the Exp activation, and P^T feeds the
    PV matmul directly (no on-chip transposes anywhere).
  - V gets a ones-column appended (stationary M=65) so the softmax
    denominators fall out of the PV matmul for free as output row 64.

Precision: the whole pipeline runs bf16 into fp32 PSUM accumulators
(measured ~3e-3 max rel err vs the 2e-2 gate).  Output partials are
written bf16 and summed on the host in fp32.

Scheduling: activations/weights are host-permuted to [128, ...] flat
layouts and fetched with a few long-line DMAs (more/smaller DMA chunks
measurably regress: each dma_start costs ~1us of sync-engine descriptor
work); softmax normalization runs off the PE stream entirely (DVE moves
the denominator row to partition 0, ACT computes 1/d = exp(-ln d) --
Ln/Exp share an activation table with the softmax Exp, and the plain
Reciprocal DVE op is ~8 cycles/element on a single partition -- then
GpSimd partition_broadcast fans it out and DVE multiplies); the output
projection is emitted in thirds as soon as the needed ctx halves exist,
plugging the attention-group boundary stalls with PE work.
"""

import os
import ml_dtypes
import numpy as np

from contextlib import ExitStack

import concourse.bass as bass
import concourse.mybir as mybir
import concourse.tile as tile
from concourse import bacc
from concourse.bass_utils import run_bass_kernel_spmd

F32 = mybir.dt.float32
BF16 = mybir.dt.bfloat16
F8 = mybir.dt.float8e4

D_MODEL = 1024
S = 2048          # query length
SK = 1152         # compacted+padded key length (9 tiles of 128)
KT = SK // 128    # key tiles
BATCH = 2
N_CORES = 8
HEADS_PER_CORE = 4
DK = 64
DO = HEADS_PER_CORE * DK  # 256 head-dims per core
MASK_BIAS = -300.0

AF = mybir.ActivationFunctionType
ALU = mybir.AluOpType


def build_program() -> bass.Bass:
    nc = bacc.Bacc("TRN2", target_bir_lowering=False, debug=False,
                   num_devices=N_CORES)

    # activations / weights arrive host-permuted: [(t p) s] -> [p (t s)]
    xq2 = nc.declare_dram_parameter("xq2", [128, 8 * S], BF16, isOutput=False)
    xk2 = nc.declare_dram_parameter("xk2", [128, 8 * SK], BF16, isOutput=False)
    xv2 = nc.declare_dram_parameter("xv2", [128, 8 * SK], BF16, isOutput=False)
    wq2 = nc.declare_dram_parameter("wq2", [128, 8 * DO], BF16, isOutput=False)
    wk2 = nc.declare_dram_parameter("wk2", [128, 8 * DO], BF16, isOutput=False)
    wv2 = nc.declare_dram_parameter("wv2", [128, 8 * DO], BF16, isOutput=False)
    woT = nc.declare_dram_parameter("woT", [DO, D_MODEL], BF16, isOutput=False)
    onesv = nc.declare_dram_parameter("onesv", [128, KT * HEADS_PER_CORE],
                                      BF16, isOutput=False)
    bq2 = nc.declare_dram_parameter("bq2", [128, 2], F32, isOutput=False)
    maskb = nc.declare_dram_parameter("maskb", [128, KT], F32, isOutput=False)
    out = nc.declare_dram_parameter("out", [S, D_MODEL], BF16, isOutput=True)
    out3 = out.rearrange("(j p) o -> p j o", p=128)

    with tile.TileContext(nc) as tc, ExitStack() as ctx, \
            nc.allow_low_precision(reason="bf16 pipeline, 2e-2 tolerance"):
        consts = ctx.enter_context(tc.tile_pool(name="consts", bufs=1))
        big = ctx.enter_context(tc.tile_pool(name="big", bufs=1))
        ppool = ctx.enter_context(tc.tile_pool(name="ppool", bufs=2))
        opool = ctx.enter_context(tc.tile_pool(name="opool", bufs=2))
        ps_mm = ctx.enter_context(tc.tile_pool(name="ps_mm", bufs=3, space="PSUM"))
        ps_ctx = ctx.enter_context(tc.tile_pool(name="ps_ctx", bufs=5, space="PSUM"))

        # ---- prefetch: one long-line DMA per tensor ----
        wv_sb = consts.tile([128, 8, DO], BF16)
        nc.sync.dma_start(wv_sb, wv2.rearrange("p (t d) -> p t d", t=8))
        xv_sb = big.tile([128, 8, SK], BF16)
        xv3 = xv2.rearrange("p (t s) -> p t s", t=8)
        for lo, hi in ((0, 384), (384, 768), (768, SK)):
            nc.sync.dma_start(xv_sb[:, :, lo:hi], xv3[:, :, lo:hi])
        wk_sb = consts.tile([128, 8, DO], BF16)
        nc.sync.dma_start(wk_sb, wk2.rearrange("p (t d) -> p t d", t=8))
        wq_sb = consts.tile([128, 8, DO], BF16)
        nc.sync.dma_start(wq_sb, wq2.rearrange("p (t d) -> p t d", t=8))
        xk_sb = big.tile([128, 8, SK], BF16)
        xk3 = xk2.rearrange("p (t s) -> p t s", t=8)
        xq_sb = big.tile([128, 8, S], BF16)
        xq3 = xq2.rearrange("p (t s) -> p t s", t=8)
        nc.sync.dma_start(xk_sb[:, :, 0:SK // 2], xk3[:, :, 0:SK // 2])
        nc.sync.dma_start(xq_sb[:, :, 0:512], xq3[:, :, 0:512])
        nc.sync.dma_start(xk_sb[:, :, SK // 2:SK], xk3[:, :, SK // 2:SK])
        for lo, hi in ((512, 1024), (1024, 1536), (1536, 2048)):
            nc.sync.dma_start(xq_sb[:, :, lo:hi], xq3[:, :, lo:hi])
        bq_sb = consts.tile([128, 2], F32)
        nc.sync.dma_start(bq_sb, bq2[:, :])
        mask_sb = consts.tile([128, KT], F32)
        nc.sync.dma_start(mask_sb, maskb[:, :])

        # ---- persistent activations ----
        qT_sb = big.tile([128, 2, S], BF16)   # Q^T: head-pair tiles on partitions
        kT_sb = big.tile([128, 2, SK], BF16)
        # normalized ctx^T per head-pair, [128 c, S] (head hh at partition
        # 64*hh) so the output projection contracts 128 partitions at once
        ctx_tiles = [big.tile([128, S], BF16, name=f"ctxT{hp}", tag=f"ctxT{hp}")
                     for hp in range(2)]
        # V tiles with the ones column (written once by a single DMA)
        v_sb = big.tile([128, KT, HEADS_PER_CORE, DK + 1], BF16)
        nc.sync.dma_start(
            v_sb[:, :, :, DK:DK + 1],
            onesv.rearrange("p (t h o) -> p t h o", t=KT, o=1))

        # ---- V projection: V[s, do] tiles ----
        for st in range(KT):
            ps = ps_mm.tile([128, 512], F32, name="ps_v", tag="mm")
            for di in range(8):
                nc.tensor.matmul(
                    ps[:, :DO],
                    lhsT=xv_sb[:, di, st * 128:(st + 1) * 128],
                    rhs=wv_sb[:, di, :],
                    start=(di == 0), stop=(di == 7))
            nc.vector.tensor_copy(
                out=v_sb[:, st, :, 0:DK],
                in_=ps[:, :DO].rearrange("p (h d) -> p h d", h=HEADS_PER_CORE))

        # ---- K^T projection (SK wide) ----
        for off, width in ((0, 512), (512, 512), (1024, 128)):
            for dt_ in range(2):
                ps = ps_mm.tile([128, 512], F32, name="ps_k", tag="mm")
                for di in range(8):
                    nc.tensor.matmul(
                        ps[:, :width],
                        lhsT=wk_sb[:, di, dt_ * 128:(dt_ + 1) * 128],
                        rhs=xk_sb[:, di, off:off + width],
                        start=(di == 0), stop=(di == 7))
                nc.vector.tensor_copy(out=kT_sb[:, dt_, off:off + width],
                                      in_=ps[:, :width])

        # ---- Q^T projection ----
        for sc in range(4):
            for dt_ in range(2):
                ps = ps_mm.tile([128, 512], F32, name="ps_q", tag="mm")
                for di in range(8):
                    nc.tensor.matmul(
                        ps, lhsT=wq_sb[:, di, dt_ * 128:(dt_ + 1) * 128],
                        rhs=xq_sb[:, di, sc * 512:(sc + 1) * 512],
                        start=(di == 0), stop=(di == 7))
                nc.scalar.activation(out=qT_sb[:, dt_, sc * 512:(sc + 1) * 512],
                                     in_=ps, func=AF.Identity,
                                     bias=bq_sb[:, dt_:dt_ + 1], scale=1.0)

        # ---- output projection weights (needed mid-kernel) ----
        wo_tiles = []
        for hp in range(2):
            wt = consts.tile([128, D_MODEL], BF16, name=f"wo{hp}", tag=f"wo{hp}")
            nc.sync.dma_start(wt, woT[hp * 128:(hp + 1) * 128, :])
            wo_tiles.append(wt)

        def attention_group(hp, qh, prev_tail=None):
            """The last key-tile's PV and the normalize chain are deferred
            into the returned closure, which the NEXT group emits right
            after its first scores batch: the PE crosses the group boundary
            straight into fresh scores while the previous tail drains on
            ACT/GpSimd/DVE."""
            ctx_ps = [[ps_ctx.tile([128, 512], F32, name="ctx_ps", tag="ctx")
                       for _ in range(2)] for _ in range(2)]
            state = {}

            def do_pv(st, pts):
                for hh in range(2):
                    for qc in range(2):
                        nc.tensor.matmul(
                            ctx_ps[hh][qc][0:DK + 1, :],
                            lhsT=v_sb[:, st, 2 * hp + hh, :],
                            rhs=pts[(hh, qc)],
                            start=(st == 0), stop=(st == KT - 1))

            def normalize():
                # off the PE stream: ACT computes 1/d = exp(-ln d) straight
                # from the PSUM denominator row (partition 64 -> 0; Ln/Exp
                # share an activation table with the softmax Exp), GpSimd
                # broadcasts partition 0 to 64 partitions, DVE multiplies.
                dens, rsbs = [], []
                for hh in range(2):
                    for qc in range(2):
                        den0 = ppool.tile([1, 512], F32, name="den0",
                                          tag="rp", bufs=4)
                        lnd = ppool.tile([1, 512], F32, name="lnd",
                                         tag="lnd", bufs=4)
                        nc.scalar.activation(out=lnd[0:1, :],
                                             in_=ctx_ps[hh][qc][DK:DK + 1, :],
                                             func=AF.Ln, scale=1.0)
                        nc.scalar.activation(out=den0[0:1, :],
                                             in_=lnd[0:1, :],
                                             func=AF.Exp, scale=-1.0)
                        dens.append(den0)
                for i in range(4):
                    r_sb = ppool.tile([64, 512], F32, name="r_sb",
                                      tag="r_sb", bufs=4)
                    nc.gpsimd.partition_broadcast(r_sb[:, :], dens[i][0:1, :])
                    rsbs.append(r_sb)
                for hh in range(2):
                    for qc in range(2):
                        qoff = qh * 1024 + qc * 512
                        nc.vector.tensor_tensor(
                            ctx_tiles[hp][64 * hh:64 * hh + 64,
                                          qoff:qoff + 512],
                            ctx_ps[hh][qc][0:DK, :],
                            rsbs[2 * hh + qc][:, :],
                            ALU.mult)

            for st in range(KT):
                pts = {}
                for hh in range(2):
                    p0 = 64 * hh
                    for qc in range(2):
                        qoff = qh * 1024 + qc * 512
                        sps = ps_mm.tile([128, 512], F32, name="sps", tag="mm")
                        nc.tensor.matmul(
                            sps,
                            lhsT=kT_sb[p0:p0 + 64, hp,
                                       st * 128:(st + 1) * 128],
                            rhs=qT_sb[p0:p0 + 64, hp, qoff:qoff + 512],
                            start=True, stop=True,
                            tile_position=(p0, 0))
                        pt = ppool.tile([128, 512], BF16, name="pt",
                                        tag="pT", bufs=12)
                        nc.scalar.activation(
                            out=pt, in_=sps, func=AF.Exp,
                            bias=mask_sb[:, st:st + 1], scale=0.125)
                        pts[(hh, qc)] = pt
                if st == 0 and prev_tail is not None:
                    prev_tail()
                if st < KT - 1:
                    do_pv(st, pts)
                else:
                    state["last_pts"] = pts

            def tail():
                do_pv(KT - 1, state["last_pts"])
                normalize()

            return tail

        def out_proj(jlo, n=8, tail=False):
            batches = [(j, 2) for j in range(jlo, jlo + n, 2)]
            if tail:
                batches = batches[:-1] + [(jlo + n - 2, 1), (jlo + n - 1, 1)]
            for j, w in batches:
                o_sb = opool.tile([128, 2, D_MODEL], BF16, name="o_sb", tag="o")
                for jj in range(w):
                    for oc in range(2):
                        ps = ps_mm.tile([128, 512], F32, name="ps_o", tag="mm")
                        so = j + jj
                        for hp in range(2):
                            nc.tensor.matmul(
                                ps,
                                lhsT=ctx_tiles[hp][:, so * 128:(so + 1) * 128],
                                rhs=wo_tiles[hp][:, oc * 512:(oc + 1) * 512],
                                start=(hp == 0), stop=(hp == 1))
                        nc.vector.tensor_copy(
                            out=o_sb[:, jj, oc * 512:(oc + 1) * 512], in_=ps)
                nc.sync.dma_start(out3[:, j:j + w, :], o_sb[:, 0:w, :])

        # query half 0: both head pairs, then its output projection rows;
        # half of it is deferred to plug the group-3/group-4 boundary
        t1 = attention_group(0, 0)
        t2 = attention_group(1, 0, prev_tail=t1)
        t3 = attention_group(0, 1, prev_tail=t2)
        out_proj(0, 4)
        t4 = attention_group(1, 1, prev_tail=t3)
        t4()
        out_proj(4, 4)
        out_proj(8, tail=True)

    nc.finalize()
    return nc


_NC_CACHE: dict = {}
LAST_RESULTS = None


def _get_program() -> bass.Bass:
    if "nc" not in _NC_CACHE:
        _NC_CACHE["nc"] = build_program()
    return _NC_CACHE["nc"]


def _flat(a, t=8):
    """[ (t p) s ] -> [ p (t s) ] host permutation, contiguous bf16."""
    tp, s_ = a.shape
    return np.ascontiguousarray(
        a.reshape(t, 128, s_).transpose(1, 0, 2).reshape(128, t * s_))


def make_in_maps(query, key_, value, mask, Wq, bq, Wk, Wv, Wo):
    bf16 = ml_dtypes.bfloat16
    in_maps = []
    # per-batch key compaction (exact: masked keys have softmax weight 0)
    kc, vc, nk = [], [], []
    for b in range(BATCH):
        idx = np.flatnonzero(mask[b, 0, 0])
        n = len(idx)
        assert n <= SK, f"mask keeps {n} keys > SK={SK}; raise SK and rebuild"
        kcb = np.zeros((SK, D_MODEL), np.float32)
        vcb = np.zeros((SK, D_MODEL), np.float32)
        kcb[:n] = key_[b][idx]
        vcb[:n] = value[b][idx]
        kc.append(kcb)
        vc.append(vcb)
        nk.append(n)
    for c in range(N_CORES):
        b, hg = divmod(c, 4)
        sl = slice(hg * DO, (hg + 1) * DO)
        pos = np.arange(SK).reshape(KT, 128).T
        maskbd = np.where(pos < nk[b], np.float32(0.0),
                          np.float32(MASK_BIAS)).astype(np.float32)
        in_maps.append({
            "xq2": _flat(query[b].T.astype(bf16)),
            "xk2": _flat(kc[b].T.astype(bf16)),
            "xv2": _flat(vc[b].T.astype(bf16)),
            "wq2": _flat(Wq[sl, :].T.astype(bf16)),
            "wk2": _flat(Wk[sl, :].T.astype(bf16)),
            "wv2": _flat(Wv[sl, :].T.astype(bf16)),
            "woT": np.ascontiguousarray(Wo[:, sl].T.astype(bf16)),
            "bq2": np.ascontiguousarray(bq[sl].reshape(2, 128).T,
                                        dtype=np.float32),
            "maskb": maskbd,
            "onesv": np.ones((128, KT * HEADS_PER_CORE), bf16),
        })
    return in_maps


def kernel(query, key_, value, mask, Wq, bq, Wk, bk, Wv, bv, Wo, bo):
    global LAST_RESULTS
    query = np.asarray(query, dtype=np.float32)
    key_ = np.asarray(key_, dtype=np.float32)
    value = np.asarray(value, dtype=np.float32)
    mask = np.asarray(mask)
    nc = _get_program()
    in_maps = make_in_maps(query, key_, value, mask,
                           np.asarray(Wq), np.asarray(bq), np.asarray(Wk),
                           np.asarray(Wv), np.asarray(Wo))
    res = run_bass_kernel_spmd(nc, in_maps, list(range(N_CORES)))
    LAST_RESULTS = res
    # host-side unshard: sum head-group partials, add bias correction row
    corr = (np.asarray(bv, dtype=np.float32) @ np.asarray(Wo, dtype=np.float32).T
            + np.asarray(bo, dtype=np.float32))
    out = np.zeros((BATCH, S, D_MODEL), np.float32)
    for c in range(N_CORES):
        out[c // 4] += np.asarray(res.results[c]["out"], dtype=np.float32)
    out += corr[None, None, :]
    return out


# revision 65
# speedup vs baseline: 1.1949x; 1.1890x over previous
"""Multi-head attention (B=2, S=2048, D=1024, H=16, dk=64) on 8 TRN2 cores.

Sharding: core c handles batch b = c//4 and head group hg = c%4 (4 heads,
256 head-dims).  Each core computes Q/K/V projections for its head slice,
attention for its 4 heads, and a partial output projection against the
matching 256-row slice of Wo.  The host sums the 4 partials per batch.

Key compaction (exact): the key mask is per (batch, key) and zeroes the
softmax weight of masked keys exactly, so the host gathers the unmasked
key/value rows (~1000 of 2048 per batch) and pads to SK=1152.  K/V
projection, scores, Exp and PV shrink by ~44% with bit-identical math
(pad columns get a -300 additive bias, exp -> 0).

Math simplifications (exact up to fp rounding):
  - bk dropped: softmax(q.(k+bk)) == softmax(q.k + const_per_row) == softmax(q.k)
  - bv dropped on device: attn rows sum to 1, so ctx = attn@V0 + bv; the
    bv term contributes the constant row bv@Wo.T, added on host with bo.
  - scores computed TRANSPOSED (S^T[k,q] = K.Q^T) so the key mask is a
    per-partition bias folded into the Exp activation, and P^T feeds the
    PV matmul directly (no on-chip transposes anywhere).
  - V gets a ones-column appended (stationary M=65) so the softmax
    denominators fall out of the PV matmul for free as output row 64.

Precision: the whole pipeline runs bf16 into fp32 PSUM accumulators
(measured ~3e-3 max rel err vs the 2e-2 gate).  Output partials are
written bf16 and summed on the host in fp32.

Scheduling: activations/weights are host-permuted to [128, ...] flat
layouts and fetched with a few long-line DMAs (more/smaller DMA chunks
measurably regress: each dma_start costs ~1us of sync-engine descriptor
work); softmax normalization runs off the PE stream entirely (DVE moves
the denominator row to partition 0, ACT computes 1/d = exp(-ln d) --
Ln/Exp share an activation table with the softmax Exp, and the plain
Reciprocal DVE op is ~8 cycles/element on a single partition -- then
GpSimd partition_broadcast fans it out and DVE multiplies); the output
projection is emitted in thirds as soon as the needed ctx halves exist,
plugging the attention-group boundary stalls with PE work.
"""

import os
import ml_dtypes
import numpy as np

from contextlib import ExitStack

import concourse.bass as bass
import concourse.mybir as mybir
import concourse.tile as tile
from concourse import bacc
from concourse.bass_utils import run_bass_kernel_spmd

F32 = mybir.dt.float32
BF16 = mybir.dt.bfloat16
F8 = mybir.dt.float8e4

D_MODEL = 1024
S = 2048          # query length
SK = 1152         # compacted+padded key length (9 tiles of 128)
KT = SK // 128    # key tiles
BATCH = 2
N_CORES = 8
HEADS_PER_CORE = 4
DK = 64
DO = HEADS_PER_CORE * DK  # 256 head-dims per core
MASK_BIAS = -300.0

AF = mybir.ActivationFunctionType
ALU = mybir.AluOpType


def build_program() -> bass.Bass:
    nc = bacc.Bacc("TRN2", target_bir_lowering=False, debug=False,
                   num_devices=N_CORES)

    # activations / weights arrive host-permuted: [(t p) s] -> [p (t s)]
    xq2 = nc.declare_dram_parameter("xq2", [128, 8 * S], BF16, isOutput=False)
    xk2 = nc.declare_dram_parameter("xk2", [128, 8 * SK], BF16, isOutput=False)
    xv2 = nc.declare_dram_parameter("xv2", [128, 8 * SK], BF16, isOutput=False)
    wq2 = nc.declare_dram_parameter("wq2", [128, 8 * DO], BF16, isOutput=False)
    wk2 = nc.declare_dram_parameter("wk2", [128, 8 * DO], BF16, isOutput=False)
    wv2 = nc.declare_dram_parameter("wv2", [128, 8 * DO], BF16, isOutput=False)
    woT = nc.declare_dram_parameter("woT", [DO, D_MODEL], BF16, isOutput=False)
    onesv = nc.declare_dram_parameter("onesv", [128, KT * HEADS_PER_CORE],
                                      BF16, isOutput=False)
    bq2 = nc.declare_dram_parameter("bq2", [128, 2], F32, isOutput=False)
    maskb = nc.declare_dram_parameter("maskb", [128, KT], F32, isOutput=False)
    out = nc.declare_dram_parameter("out", [S, D_MODEL], BF16, isOutput=True)
    out3 = out.rearrange("(j p) o -> p j o", p=128)

    with tile.TileContext(nc) as tc, ExitStack() as ctx, \
            nc.allow_low_precision(reason="bf16 pipeline, 2e-2 tolerance"):
        consts = ctx.enter_context(tc.tile_pool(name="consts", bufs=1))
        big = ctx.enter_context(tc.tile_pool(name="big", bufs=1))
        ppool = ctx.enter_context(tc.tile_pool(name="ppool", bufs=2))
        opool = ctx.enter_context(tc.tile_pool(name="opool", bufs=2))
        ps_mm = ctx.enter_context(tc.tile_pool(name="ps_mm", bufs=3, space="PSUM"))
        ps_ctx = ctx.enter_context(tc.tile_pool(name="ps_ctx", bufs=5, space="PSUM"))

        # ---- prefetch: one long-line DMA per tensor ----
        wv_sb = consts.tile([128, 8, DO], BF16)
        nc.sync.dma_start(wv_sb, wv2.rearrange("p (t d) -> p t d", t=8))
        xv_sb = big.tile([128, 8, SK], BF16)
        xv3 = xv2.rearrange("p (t s) -> p t s", t=8)
        for lo, hi in ((0, 384), (384, 768), (768, SK)):
            nc.sync.dma_start(xv_sb[:, :, lo:hi], xv3[:, :, lo:hi])
        wk_sb = consts.tile([128, 8, DO], BF16)
        nc.sync.dma_start(wk_sb, wk2.rearrange("p (t d) -> p t d", t=8))
        wq_sb = consts.tile([128, 8, DO], BF16)
        nc.sync.dma_start(wq_sb, wq2.rearrange("p (t d) -> p t d", t=8))
        xk_sb = big.tile([128, 8, SK], BF16)
        xk3 = xk2.rearrange("p (t s) -> p t s", t=8)
        xq_sb = big.tile([128, 8, S], BF16)
        xq3 = xq2.rearrange("p (t s) -> p t s", t=8)
        nc.sync.dma_start(xk_sb[:, :, 0:SK // 2], xk3[:, :, 0:SK // 2])
        nc.sync.dma_start(xq_sb[:, :, 0:512], xq3[:, :, 0:512])
        nc.sync.dma_start(xk_sb[:, :, SK // 2:SK], xk3[:, :, SK // 2:SK])
        for lo, hi in ((512, 1024), (1024, 1536), (1536, 2048)):
            nc.sync.dma_start(xq_sb[:, :, lo:hi], xq3[:, :, lo:hi])
        bq_sb = consts.tile([128, 2], F32)
        nc.sync.dma_start(bq_sb, bq2[:, :])
        mask_sb = consts.tile([128, KT], F32)
        nc.sync.dma_start(mask_sb, maskb[:, :])

        # ---- persistent activations ----
        qT_sb = big.tile([128, 2, S], BF16)   # Q^T: head-pair tiles on partitions
        kT_sb = big.tile([128, 2, SK], BF16)
        # normalized ctx^T per head-pair, [128 c, S] (head hh at partition
        # 64*hh) so the output projection contracts 128 partitions at once
        ctx_tiles = [big.tile([128, S], BF16, name=f"ctxT{hp}", tag=f"ctxT{hp}")
                     for hp in range(2)]
        # V tiles with the ones column (written once by a single DMA)
        v_sb = big.tile([128, KT, HEADS_PER_CORE, DK + 1], BF16)
        nc.sync.dma_start(
            v_sb[:, :, :, DK:DK + 1],
            onesv.rearrange("p (t h o) -> p t h o", t=KT, o=1))

        # ---- V projection: V[s, do] tiles ----
        for st in range(KT):
            ps = ps_mm.tile([128, 512], F32, name="ps_v", tag="mm")
            for di in range(8):
                nc.tensor.matmul(
                    ps[:, :DO],
                    lhsT=xv_sb[:, di, st * 128:(st + 1) * 128],
                    rhs=wv_sb[:, di, :],
                    start=(di == 0), stop=(di == 7))
            nc.vector.tensor_copy(
                out=v_sb[:, st, :, 0:DK],
                in_=ps[:, :DO].rearrange("p (h d) -> p h d", h=HEADS_PER_CORE))

        # ---- K^T projection (SK wide) ----
        for off, width in ((0, 512), (512, 512), (1024, 128)):
            for dt_ in range(2):
                ps = ps_mm.tile([128, 512], F32, name="ps_k", tag="mm")
                for di in range(8):
                    nc.tensor.matmul(
                        ps[:, :width],
                        lhsT=wk_sb[:, di, dt_ * 128:(dt_ + 1) * 128],
                        rhs=xk_sb[:, di, off:off + width],
                        start=(di == 0), stop=(di == 7))
                nc.vector.tensor_copy(out=kT_sb[:, dt_, off:off + width],
                                      in_=ps[:, :width])

        # ---- Q^T projection ----
        for sc in range(4):
            for dt_ in range(2):
                ps = ps_mm.tile([128, 512], F32, name="ps_q", tag="mm")
                for di in range(8):
                    nc.tensor.matmul(
                        ps, lhsT=wq_sb[:, di, dt_ * 128:(dt_ + 1) * 128],
                        rhs=xq_sb[:, di, sc * 512:(sc + 1) * 512],
                        start=(di == 0), stop=(di == 7))
                nc.scalar.activation(out=qT_sb[:, dt_, sc * 512:(sc + 1) * 512],
                                     in_=ps, func=AF.Identity,
                                     bias=bq_sb[:, dt_:dt_ + 1], scale=1.0)

        # ---- output projection weights (needed mid-kernel) ----
        wo_tiles = []
        for hp in range(2):
            wt = consts.tile([128, D_MODEL], BF16, name=f"wo{hp}", tag=f"wo{hp}")
            nc.sync.dma_start(wt, woT[hp * 128:(hp + 1) * 128, :])
            wo_tiles.append(wt)

        def attention_group(hp, qh, prev_tail=None):
            """The last key-tile's PV and the normalize chain are deferred
            into the returned closure, which the NEXT group emits right
            after its first scores batch: the PE crosses the group boundary
            straight into fresh scores while the previous tail drains on
            ACT/GpSimd/DVE."""
            ctx_ps = [[ps_ctx.tile([128, 512], F32, name="ctx_ps", tag="ctx")
                       for _ in range(2)] for _ in range(2)]
            state = {}

            def do_pv(st, pts):
                for hh in range(2):
                    for qc in range(2):
                        nc.tensor.matmul(
                            ctx_ps[hh][qc][0:DK + 1, :],
                            lhsT=v_sb[:, st, 2 * hp + hh, :],
                            rhs=pts[(hh, qc)],
                            start=(st == 0), stop=(st == KT - 1))

            def normalize():
                # off the PE stream, ACT-minimal: DVE gathers the four PSUM
                # denominator rows onto partitions 0-3 of one tile, ACT
                # computes 1/d = exp(-ln d) for all four in ONE op pair
                # (ACT cost scales with free size only, not partitions),
                # DVE scatters each back to a partition-0 tile for the
                # GpSimd broadcast, DVE multiplies.
                gat = ppool.tile([128, 512], F32, name="gat", tag="gat",
                                 bufs=2)
                nc.vector.memset(gat, 1.0)
                i = 0
                for hh in range(2):
                    for qc in range(2):
                        nc.vector.tensor_copy(
                            out=gat[32 * i:32 * i + 1, :],
                            in_=ctx_ps[hh][qc][DK:DK + 1, :])
                        i += 1
                lnr = ppool.tile([128, 512], F32, name="lnr", tag="lnr",
                                 bufs=2)
                nc.scalar.activation(out=lnr, in_=gat,
                                     func=AF.Ln, scale=1.0)
                rec = ppool.tile([128, 512], F32, name="rec", tag="rec",
                                 bufs=2)
                nc.scalar.activation(out=rec, in_=lnr,
                                     func=AF.Exp, scale=-1.0)
                dens, rsbs = [], []
                for i in range(4):
                    d0 = ppool.tile([1, 512], F32, name="d0", tag="rs1",
                                    bufs=4)
                    nc.vector.tensor_copy(out=d0[0:1, :],
                                          in_=rec[32 * i:32 * i + 1, :])
                    dens.append(d0)
                for i in range(4):
                    r_sb = ppool.tile([64, 512], F32, name="r_sb",
                                      tag="r_sb", bufs=4)
                    nc.gpsimd.partition_broadcast(r_sb[:, :], dens[i][0:1, :])
                    rsbs.append(r_sb)
                for hh in range(2):
                    for qc in range(2):
                        qoff = qh * 1024 + qc * 512
                        nc.vector.tensor_tensor(
                            ctx_tiles[hp][64 * hh:64 * hh + 64,
                                          qoff:qoff + 512],
                            ctx_ps[hh][qc][0:DK, :],
                            rsbs[2 * hh + qc][:, :],
                            ALU.mult)

            for st in range(KT):
                pts = {}
                for hh in range(2):
                    p0 = 64 * hh
                    for qc in range(2):
                        qoff = qh * 1024 + qc * 512
                        sps = ps_mm.tile([128, 512], F32, name="sps", tag="mm")
                        nc.tensor.matmul(
                            sps,
                            lhsT=kT_sb[p0:p0 + 64, hp,
                                       st * 128:(st + 1) * 128],
                            rhs=qT_sb[p0:p0 + 64, hp, qoff:qoff + 512],
                            start=True, stop=True,
                            tile_position=(p0, 0))
                        pt = ppool.tile([128, 512], BF16, name="pt",
                                        tag="pT", bufs=12)
                        nc.scalar.activation(
                            out=pt, in_=sps, func=AF.Exp,
                            bias=mask_sb[:, st:st + 1], scale=0.125)
                        pts[(hh, qc)] = pt
                if st == 0 and prev_tail is not None:
                    prev_tail()
                if st < KT - 1:
                    do_pv(st, pts)
                else:
                    state["last_pts"] = pts

            def tail():
                do_pv(KT - 1, state["last_pts"])
                normalize()

            return tail

        def out_proj(jlo, n=8, tail=False):
            batches = [(j, 2) for j in range(jlo, jlo + n, 2)]
            if tail:
                batches = batches[:-1] + [(jlo + n - 2, 1), (jlo + n - 1, 1)]
            for j, w in batches:
                o_sb = opool.tile([128, 2, D_MODEL], BF16, name="o_sb", tag="o")
                for jj in range(w):
                    for oc in range(2):
                        ps = ps_mm.tile([128, 512], F32, name="ps_o", tag="mm")
                        so = j + jj
                        for hp in range(2):
                            nc.tensor.matmul(
                                ps,
                                lhsT=ctx_tiles[hp][:, so * 128:(so + 1) * 128],
                                rhs=wo_tiles[hp][:, oc * 512:(oc + 1) * 512],
                                start=(hp == 0), stop=(hp == 1))
                        nc.vector.tensor_copy(
                            out=o_sb[:, jj, oc * 512:(oc + 1) * 512], in_=ps)
                nc.sync.dma_start(out3[:, j:j + w, :], o_sb[:, 0:w, :])

        # query half 0: both head pairs, then its output projection rows;
        # half of it is deferred to plug the group-3/group-4 boundary
        t1 = attention_group(0, 0)
        t2 = attention_group(1, 0, prev_tail=t1)
        t3 = attention_group(0, 1, prev_tail=t2)
        out_proj(0, 4)
        t4 = attention_group(1, 1, prev_tail=t3)
        t4()
        out_proj(4, 4)
        out_proj(8, tail=True)

    nc.finalize()
    return nc


_NC_CACHE: dict = {}
LAST_RESULTS = None


def _get_program() -> bass.Bass:
    if "nc" not in _NC_CACHE:
        _NC_CACHE["nc"] = build_program()
    return _NC_CACHE["nc"]


def _flat(a, t=8):
    """[ (t p) s ] -> [ p (t s) ] host permutation, contiguous bf16."""
    tp, s_ = a.shape
    return np.ascontiguousarray(
        a.reshape(t, 128, s_).transpose(1, 0, 2).reshape(128, t * s_))


def make_in_maps(query, key_, value, mask, Wq, bq, Wk, Wv, Wo):
    bf16 = ml_dtypes.bfloat16
    in_maps = []
    # per-batch key compaction (exact: masked keys have softmax weight 0)
    kc, vc, nk = [], [], []
    for b in range(BATCH):
        idx = np.flatnonzero(mask[b, 0, 0])
        n = len(idx)
        assert n <= SK, f"mask keeps {n} keys > SK={SK}; raise SK and rebuild"
        kcb = np.zeros((SK, D_MODEL), np.float32)
        vcb = np.zeros((SK, D_MODEL), np.float32)
        kcb[:n] = key_[b][idx]
        vcb[:n] = value[b][idx]
        kc.append(kcb)
        vc.append(vcb)
        nk.append(n)
    for c in range(N_CORES):
        b, hg = divmod(c, 4)
        sl = slice(hg * DO, (hg + 1) * DO)
        pos = np.arange(SK).reshape(KT, 128).T
        maskbd = np.where(pos < nk[b], np.float32(0.0),
                          np.float32(MASK_BIAS)).astype(np.float32)
        in_maps.append({
            "xq2": _flat(query[b].T.astype(bf16)),
            "xk2": _flat(kc[b].T.astype(bf16)),
            "xv2": _flat(vc[b].T.astype(bf16)),
            "wq2": _flat(Wq[sl, :].T.astype(bf16)),
            "wk2": _flat(Wk[sl, :].T.astype(bf16)),
            "wv2": _flat(Wv[sl, :].T.astype(bf16)),
            "woT": np.ascontiguousarray(Wo[:, sl].T.astype(bf16)),
            "bq2": np.ascontiguousarray(bq[sl].reshape(2, 128).T,
                                        dtype=np.float32),
            "maskb": maskbd,
            "onesv": np.ones((128, KT * HEADS_PER_CORE), bf16),
        })
    return in_maps


def kernel(query, key_, value, mask, Wq, bq, Wk, bk, Wv, bv, Wo, bo):
    global LAST_RESULTS
    query = np.asarray(query, dtype=np.float32)
    key_ = np.asarray(key_, dtype=np.float32)
    value = np.asarray(value, dtype=np.float32)
    mask = np.asarray(mask)
    nc = _get_program()
    in_maps = make_in_maps(query, key_, value, mask,
                           np.asarray(Wq), np.asarray(bq), np.asarray(Wk),
                           np.asarray(Wv), np.asarray(Wo))
    res = run_bass_kernel_spmd(nc, in_maps, list(range(N_CORES)))
    LAST_RESULTS = res
    # host-side unshard: sum head-group partials, add bias correction row
    corr = (np.asarray(bv, dtype=np.float32) @ np.asarray(Wo, dtype=np.float32).T
            + np.asarray(bo, dtype=np.float32))
    out = np.zeros((BATCH, S, D_MODEL), np.float32)
    for c in range(N_CORES):
        out[c // 4] += np.asarray(res.results[c]["out"], dtype=np.float32)
    out += corr[None, None, :]
    return out
